# revision 11
# baseline (speedup 1.0000x reference)
"""Trainium2 Bass kernel for nn_AttentionRevisedDecoderRNN.

Computation (batch=1, seq=6):
  attn_w  = softmax(attn_in @ W_attn.T + b_attn, axis=hidden)      # (6, 8192)
  applied = attn_w @ enc                                           # (6, 4114)  <- memory bound
  x       = [one_hot, applied]; LSTM(hidden=6) over 6 steps
  y       = relu(relu(hs @ W_out11.T + b11) @ W_out12.T + b12); log_softmax

Strategy: shard enc + W_attn along hidden (8192 -> 1024/core) over 8 cores.
Each core computes exp(logits) for its hidden shard (softmax numerators),
contracts them against its enc shard into a partial appliedT, then folds the
LSTM input projection (applied @ W_ih[:,6:].T) and the softmax denominator s
into a tiny (25,6) partial that is AllReduced.  After the AllReduce every
core runs the (tiny) LSTM + output head redundantly; core 0's output is used.
"""

import sys
import numpy as np
import ml_dtypes

BF = ml_dtypes.bfloat16

for _p in ("/opt/trn_rl_repo",):
    if _p not in sys.path:
        sys.path.insert(0, _p)

from concourse import bass, bacc, tile, mybir
from concourse.bass_utils import run_bass_kernel_spmd

F32 = mybir.dt.float32
BF16 = mybir.dt.bfloat16
NCORES = 8
HIDDEN = 8192
NS = HIDDEN // NCORES        # hidden shard per core = 1024
HC = NS // 128               # h-chunks per core = 8
KDIM = 4114                  # enc free width
NT = (KDIM + 127) // 128     # 33 n-tiles (last = 18 rows)
SEQ = 6
G4 = 24                      # 4*OUT gates

_cached = {}


def build_program(debug_taps=False):
    nc = bacc.Bacc("TRN2", target_bir_lowering=False, debug=False,
                   enable_asserts=False, num_devices=NCORES)

    # ---- kernel I/O ----
    enc_d = nc.dram_tensor("enc", [NS, KDIM], F32, kind="ExternalInput")
    wta_d = nc.dram_tensor("wta", [19, NS], BF16, kind="ExternalInput")
    ait_d = nc.dram_tensor("ait", [19, 6], BF16, kind="ExternalInput")
    w2t_d = nc.dram_tensor("w2t", [128, NT * G4], BF16, kind="ExternalInput")
    gob_d = nc.dram_tensor("gob", [6, G4], F32, kind="ExternalInput")
    whht_d = nc.dram_tensor("whht", [6, G4], BF16, kind="ExternalInput")
    h0t_d = nc.dram_tensor("h0t", [6, 1], BF16, kind="ExternalInput")
    c0_d = nc.dram_tensor("c0", [1, 6], F32, kind="ExternalInput")
    w11t_d = nc.dram_tensor("w11t", [6, HIDDEN], BF16, kind="ExternalInput")
    b11r_d = nc.dram_tensor("b11r", [128, 384], F32, kind="ExternalInput")
    w12t_d = nc.dram_tensor("w12t", [128, 384], BF16, kind="ExternalInput")
    b12r_d = nc.dram_tensor("b12r", [6, 6], F32, kind="ExternalInput")
    iden_d = nc.dram_tensor("iden", [128, 128], F32, kind="ExternalInput")

    logp_d = nc.dram_tensor("logp", [6, 6], F32, kind="ExternalOutput")
    hout_d = nc.dram_tensor("hout", [1, 6], F32, kind="ExternalOutput")
    cout_d = nc.dram_tensor("cout", [1, 6], F32, kind="ExternalOutput")
    warm_d = nc.dram_tensor("warm", [1, 2], F32, kind="ExternalOutput")
    if debug_taps:
        det_d = nc.dram_tensor("d_et", [128, HC * 6], F32, kind="ExternalOutput")
        dat_d = nc.dram_tensor("d_at", [128, NT * 6], F32, kind="ExternalOutput")
        dred_d = nc.dram_tensor("d_red", [25, 6], F32, kind="ExternalOutput")
        dgt_d = nc.dram_tensor("d_gt", [G4, 6], F32, kind="ExternalOutput")
        dhs_d = nc.dram_tensor("d_hs", [6, SEQ], F32, kind="ExternalOutput")
        dy1_d = nc.dram_tensor("d_y1", [128, 384], F32, kind="ExternalOutput")
        dpay_d = nc.dram_tensor("d_pay", [25, 6], F32, kind="ExternalOutput")

    with tile.TileContext(nc) as tc:
        with (
            tc.tile_pool(name="consts", bufs=1) as cp,
            tc.tile_pool(name="encp", bufs=3) as encp,
            tc.tile_pool(name="work", bufs=2) as wp,
            tc.tile_pool(name="ps1", bufs=1, space=bass.MemorySpace.PSUM) as ps1,
            tc.tile_pool(name="dram", bufs=1, space="DRAM") as dp,
        ):
            # ---- load constants ----
            wta = cp.tile([19, NS], BF16)
            nc.sync.dma_start(wta[:], wta_d[:])
            ait = cp.tile([19, 6], BF16)
            nc.sync.dma_start(ait[:], ait_d[:])
            w2t = cp.tile([128, NT * G4], BF16)
            nc.sync.dma_start(w2t[:], w2t_d[:])
            gob = cp.tile([6, G4], F32)
            nc.sync.dma_start(gob[:], gob_d[:])
            whht = cp.tile([6, G4], BF16)
            nc.sync.dma_start(whht[:], whht_d[:])
            h0t = cp.tile([6, 1], BF16)
            nc.sync.dma_start(h0t[:], h0t_d[:])
            c0 = cp.tile([1, 6], F32)
            nc.sync.dma_start(c0[:], c0_d[:])
            w11t = cp.tile([6, HIDDEN], BF16)
            nc.sync.dma_start(w11t[:], w11t_d[:])
            b11r = cp.tile([128, 384], F32)
            nc.sync.dma_start(b11r[:], b11r_d[:])
            w12t = cp.tile([128, 384], BF16)
            nc.sync.dma_start(w12t[:], w12t_d[:])
            b12r = cp.tile([6, 6], F32)
            nc.sync.dma_start(b12r[:], b12r_d[:])
            iden = cp.tile([128, 128], F32)
            nc.sync.dma_start(iden[:], iden_d[:])
            ones = cp.tile([128, 1], BF16)
            nc.vector.memset(ones[:], 1.0)

            # warm up the collectives firmware so the real AllReduce
            # does not pay the cold ncfw trigger latency
            warm_sb = cp.tile([1, 2], F32)
            nc.vector.memset(warm_sb[:], 0.0)
            war_in = dp.tile([1, 2], F32)
            war_out = dp.tile([1, 2], F32)
            nc.sync.dma_start(war_in[:], warm_sb[:])
            nc.gpsimd.collective_compute(
                "AllReduce", mybir.AluOpType.add,
                replica_groups=[list(range(NCORES))],
                ins=[war_in.opt()], outs=[war_out.opt()],
            )
            nc.sync.dma_start(warm_d[:], war_out[:])

            # ---- attention logits (transposed layout) + exp ----
            # lt[:, 6m:6m+6] = W_attn_shard[128m:128m+128, :] @ attn_in.T + b  (bias folded via ones row)
            lt = ps1.tile([128, HC * 6], F32, tag="pa")
            for m in range(HC):
                nc.tensor.matmul(lt[:, 6 * m:6 * m + 6],
                                 wta[:, 128 * m:128 * (m + 1)], ait[:],
                                 start=True, stop=True)
            et = cp.tile([128, HC * 6], BF16)   # exp(logits), chunked transposed
            nc.scalar.activation(et[:], lt[:], mybir.ActivationFunctionType.Exp)

            # ---- big matmul: appliedT[n, r] = sum_h enc[h, n] * et[h, r] ----
            at_ps = ps1.tile([128, NT * 6], F32, tag="pa")
            s_ps = ps1.tile([1, 6], F32, tag="sp")       # partial softmax denominator
            for m in range(HC):
                encf = encp.tile([128, KDIM], F32, tag="encf")
                nc.sync.dma_start(encf[:], enc_d[128 * m:128 * (m + 1), :])
                enct = encp.tile([128, KDIM], BF16, tag="enct")
                nc.vector.tensor_copy(enct[:], encf[:])
                etm = et[:, 6 * m:6 * m + 6]
                for j in range(NT):
                    w = 128 if j < NT - 1 else KDIM - 128 * (NT - 1)
                    # start=True only on the very first matmul into this bank:
                    # PE's has_written clear is bank-wide, so later groups must
                    # rely on "overwrite where bit is clear" for their first
                    # write and accumulate afterwards.
                    nc.tensor.matmul(at_ps[0:w, 6 * j:6 * j + 6],
                                     enct[:, 128 * j:128 * j + w], etm,
                                     start=(m == 0 and j == 0),
                                     stop=(m == HC - 1 and j == NT - 1))
                nc.tensor.matmul(s_ps[:], ones[:], etm,
                                 start=(m == 0), stop=(m == HC - 1))

            at_sb = cp.tile([128, NT * 6], BF16)
            nc.vector.tensor_copy(at_sb[:], at_ps[:])

            # ---- partial gate pre-activations: gT[j, r] = sum_n appliedT[n,r] W2T[n,j] ----
            gt_ps = ps1.tile([G4, 6], F32, tag="gt")
            for j in range(NT):
                w = 128 if j < NT - 1 else KDIM - 128 * (NT - 1)
                nc.tensor.matmul(gt_ps[:], w2t[0:w, G4 * j:G4 * (j + 1)],
                                 at_sb[0:w, 6 * j:6 * j + 6],
                                 start=(j == 0), stop=(j == NT - 1))

            g24 = cp.tile([G4, 6], F32)
            nc.vector.tensor_copy(g24[:], gt_ps[:])
            s_sb = cp.tile([1, 6], F32)
            nc.vector.tensor_copy(s_sb[:], s_ps[:])

            # ---- AllReduce of the (25, 6) partial ----
            ar_in = dp.tile([25, 6], F32)
            ar_out = dp.tile([25, 6], F32)
            nc.sync.dma_start(ar_in[0:G4, :], g24[:])
            nc.sync.dma_start(ar_in[G4:G4 + 1, :], s_sb[:])
            nc.gpsimd.collective_compute(
                "AllReduce", mybir.AluOpType.add,
                replica_groups=[list(range(NCORES))],
                ins=[ar_in.opt()], outs=[ar_out.opt()],
            )
            red = cp.tile([25, 6], F32)
            nc.sync.dma_start(red[:], ar_out[:])

            # ---- finish gates: (6,24) row layout ----
            redT_ps = ps1.tile([6, 25], F32, tag="sp")
            nc.tensor.transpose(redT_ps[:], red[:], iden[0:25, 0:25])
            redT = cp.tile([6, 25], F32)
            nc.vector.tensor_copy(redT[:], redT_ps[:])
            inv_s = cp.tile([6, 1], F32)
            nc.vector.reciprocal(inv_s[:], redT[:, G4:G4 + 1])
            grow = cp.tile([6, G4], F32)
            # grow = redT[:, :24] * inv_s + gob
            nc.vector.scalar_tensor_tensor(grow[:], redT[:, 0:G4], inv_s[:],
                                           gob[:], mybir.AluOpType.mult,
                                           mybir.AluOpType.add)
            gT_ps = ps1.tile([G4, 6], F32, tag="gt")
            nc.tensor.transpose(gT_ps[:], grow[:], iden[0:6, 0:6])
            gT = cp.tile([G4, 6], F32)
            nc.vector.tensor_copy(gT[:], gT_ps[:])

            # ---- LSTM (6 steps, tiny) ----
            hsT = cp.tile([6, SEQ], BF16)
            h_prev_t = h0t
            c_prev = c0
            h_new = None
            c_new = None
            for t in range(SEQ):
                wh_ps = ps1.tile([G4, 1], F32, tag="wh")
                nc.tensor.matmul(wh_ps[:], whht[:], h_prev_t[:],
                                 start=True, stop=True)
                gcol = wp.tile([G4, 1], F32)
                nc.vector.tensor_add(gcol[:], gT[:, t:t + 1], wh_ps[:])
                gr_ps = ps1.tile([1, G4], F32, tag="gr")
                nc.tensor.transpose(gr_ps[:], gcol[:], iden[0:G4, 0:G4])
                sig = wp.tile([1, G4], F32)
                nc.scalar.activation(sig[:], gr_ps[:],
                                     mybir.ActivationFunctionType.Sigmoid)
                tng = wp.tile([1, 6], F32)
                nc.scalar.activation(tng[:], gr_ps[:, 12:18],
                                     mybir.ActivationFunctionType.Tanh)
                m1 = wp.tile([1, 6], F32)
                nc.vector.tensor_mul(m1[:], sig[:, 6:12], c_prev[:])
                m2 = wp.tile([1, 6], F32)
                nc.vector.tensor_mul(m2[:], sig[:, 0:6], tng[:])
                c_new = wp.tile([1, 6], F32)
                nc.vector.tensor_add(c_new[:], m1[:], m2[:])
                tc_ = wp.tile([1, 6], F32)
                nc.scalar.activation(tc_[:], c_new[:],
                                     mybir.ActivationFunctionType.Tanh)
                h_new = wp.tile([1, 6], F32)
                nc.vector.tensor_mul(h_new[:], sig[:, 18:24], tc_[:])
                ht_ps = ps1.tile([6, 1], F32, tag="ht")
                nc.tensor.transpose(ht_ps[:], h_new[:], iden[0:1, 0:1])
                nc.vector.tensor_copy(hsT[:, t:t + 1], ht_ps[:])
                h_prev_t = hsT[:, t:t + 1]
                c_prev = c_new

            nc.sync.dma_start(hout_d[:], h_new[:])
            nc.sync.dma_start(cout_d[:], c_new[:])

            # ---- output head ----
            y1_ps = ps1.tile([128, 384], F32, tag="pa")
            for cch in range(64):
                nc.tensor.matmul(y1_ps[:, 6 * cch:6 * cch + 6],
                                 w11t[:, 128 * cch:128 * (cch + 1)], hsT[:],
                                 start=True, stop=True)
            y1b = cp.tile([128, 384], F32)
            nc.vector.tensor_add(y1b[:], y1_ps[:], b11r[:])
            y1t = cp.tile([128, 384], BF16)
            nc.vector.tensor_relu(y1t[:], y1b[:])

            y2_ps = ps1.tile([6, 6], F32, tag="sp")
            for cch in range(64):
                nc.tensor.matmul(y2_ps[:], y1t[:, 6 * cch:6 * cch + 6],
                                 w12t[:, 6 * cch:6 * cch + 6],
                                 start=(cch == 0), stop=(cch == 63))
            y2b = cp.tile([6, 6], F32)
            nc.vector.tensor_add(y2b[:], y2_ps[:], b12r[:])
            y2r = cp.tile([6, 6], F32)
            nc.vector.tensor_relu(y2r[:], y2b[:])

            # ---- log_softmax over free axis ----
            mx = cp.tile([6, 1], F32)
            nc.vector.tensor_reduce(mx[:], y2r[:], mybir.AxisListType.X,
                                    mybir.AluOpType.max)
            nmx = cp.tile([6, 1], F32)
            nc.vector.tensor_scalar_mul(nmx[:], mx[:], -1.0)
            e = cp.tile([6, 6], F32)
            se = cp.tile([6, 1], F32)
            nc.scalar.activation(e[:], y2r[:], mybir.ActivationFunctionType.Exp,
                                 bias=nmx[:], accum_out=se[:])
            lse = cp.tile([6, 1], F32)
            nc.scalar.activation(lse[:], se[:], mybir.ActivationFunctionType.Ln)
            shift = cp.tile([6, 1], F32)
            nc.vector.tensor_sub(shift[:], nmx[:], lse[:])
            logp_sb = cp.tile([6, 6], F32)
            nc.vector.tensor_scalar_add(logp_sb[:], y2r[:], shift[:])
            nc.sync.dma_start(logp_d[:], logp_sb[:])

            if debug_taps:
                nc.sync.dma_start(det_d[:], et[:])
                nc.sync.dma_start(dat_d[:], at_sb[:])
                nc.sync.dma_start(dred_d[:], red[:])
                nc.sync.dma_start(dgt_d[:], gT[:])
                nc.sync.dma_start(dhs_d[:], hsT[:])
                nc.sync.dma_start(dy1_d[:], y1t[:])
                nc.sync.dma_start(dpay_d[0:G4, :], g24[:])
                nc.sync.dma_start(dpay_d[G4:G4 + 1, :], s_sb[:])

    nc.compile()
    return nc


def prep_inputs(inp, hn, cn, encoder_outputs, W_attn, b_attn, W_ih, W_hh,
                b_ih, b_hh, W_out11, b_out11, W_out12, b_out12):
    f32 = np.float32
    inp = np.asarray(inp).astype(np.int64)
    hn = np.asarray(hn, f32).reshape(6)
    cn = np.asarray(cn, f32).reshape(6)
    enc2d = np.asarray(encoder_outputs, f32).reshape(HIDDEN, KDIM)
    W_attn = np.asarray(W_attn, f32)
    b_attn = np.asarray(b_attn, f32)
    W_ih = np.asarray(W_ih, f32)
    W_hh = np.asarray(W_hh, f32)
    b_ih = np.asarray(b_ih, f32)
    b_hh = np.asarray(b_hh, f32)
    W_out11 = np.asarray(W_out11, f32)
    b_out11 = np.asarray(b_out11, f32)
    W_out12 = np.asarray(W_out12, f32)
    b_out12 = np.asarray(b_out12, f32)

    oh = np.eye(6, dtype=f32)[inp]                                   # (6,6)
    attn_in = np.concatenate(
        [oh, np.broadcast_to(hn, (6, 6)), np.broadcast_to(cn, (6, 6))], axis=1)
    ait = np.ascontiguousarray(
        np.concatenate([attn_in.T, np.ones((1, 6), f32)], axis=0)).astype(BF)

    W2T = W_ih[:, 6:].T                                              # (4114,24)
    W2T_pad = np.zeros((NT * 128, G4), f32)
    W2T_pad[:KDIM] = W2T
    w2t = np.ascontiguousarray(
        W2T_pad.reshape(NT, 128, G4).transpose(1, 0, 2)
        .reshape(128, NT * G4)).astype(BF)

    gob = np.ascontiguousarray(oh @ W_ih[:, :6].T + b_ih + b_hh)     # (6,24)
    whht = np.ascontiguousarray(W_hh.T).astype(BF)                   # (6,24)
    h0t = np.ascontiguousarray(hn.reshape(6, 1)).astype(BF)
    c0 = np.ascontiguousarray(cn.reshape(1, 6))
    w11t = np.ascontiguousarray(W_out11.T).astype(BF)                # (6,8192)
    b11r = np.ascontiguousarray(
        np.repeat(b_out11.reshape(64, 128).T.reshape(128, 64, 1), 6,
                  axis=2).reshape(128, 384))
    w12t = np.ascontiguousarray(
        W_out12.T.reshape(64, 128, 6).transpose(1, 0, 2)
        .reshape(128, 384)).astype(BF)
    b12r = np.ascontiguousarray(np.broadcast_to(b_out12, (6, 6)))
    iden = np.eye(128, dtype=f32)

    shared = dict(ait=ait, w2t=w2t, gob=gob, whht=whht, h0t=h0t, c0=c0,
                  w11t=w11t, b11r=b11r, w12t=w12t, b12r=b12r, iden=iden)
    in_maps = []
    for c in range(NCORES):
        lo = c * NS
        wta = np.ascontiguousarray(np.concatenate(
            [W_attn[lo:lo + NS].T, b_attn[None, lo:lo + NS]],
            axis=0)).astype(BF)
        m = dict(shared)
        m["enc"] = enc2d[lo:lo + NS]
        m["wta"] = wta
        in_maps.append(m)
    return in_maps


def kernel(**inputs):
    if "nc" not in _cached:
        _cached["nc"] = build_program()
    nc = _cached["nc"]
    in_maps = prep_inputs(**inputs)
    res = run_bass_kernel_spmd(nc, in_maps, core_ids=list(range(NCORES)))
    out = res.results[0]
    logp = np.asarray(out["logp"], np.float32)
    h = np.asarray(out["hout"], np.float32).reshape(1, 1, 6)
    c = np.asarray(out["cout"], np.float32).reshape(1, 1, 6)
    return (logp, h, c)


# revision 12
# speedup vs baseline: 1.1039x; 1.1039x over previous
"""Trainium2 Bass kernel for nn_AttentionRevisedDecoderRNN.

Computation (batch=1, seq=6):
  attn_w  = softmax(attn_in @ W_attn.T + b_attn, axis=hidden)      # (6, 8192)
  applied = attn_w @ enc                                           # (6, 4114)  <- memory bound
  x       = [one_hot, applied]; LSTM(hidden=6) over 6 steps
  y       = relu(relu(hs @ W_out11.T + b11) @ W_out12.T + b12); log_softmax

Strategy: shard enc + W_attn along hidden (8192 -> 1024/core) over 8 cores.
Each core computes exp(logits) for its hidden shard (softmax numerators),
contracts them against its enc shard into a partial appliedT, then folds the
LSTM input projection (applied @ W_ih[:,6:].T) and the softmax denominator s
into a tiny (25,6) partial that is AllReduced.  After the AllReduce every
core runs the (tiny) LSTM + output head redundantly; core 0's output is used.
"""

import sys
import numpy as np
import ml_dtypes

BF = ml_dtypes.bfloat16

for _p in ("/opt/trn_rl_repo",):
    if _p not in sys.path:
        sys.path.insert(0, _p)

from concourse import bass, bacc, tile, mybir
from concourse.bass_utils import run_bass_kernel_spmd

F32 = mybir.dt.float32
BF16 = mybir.dt.bfloat16
NCORES = 8
HIDDEN = 8192
NS = HIDDEN // NCORES        # hidden shard per core = 1024
HC = NS // 128               # h-chunks per core = 8
KDIM = 4114                  # enc free width
NT = (KDIM + 127) // 128     # 33 n-tiles (last = 18 rows)
SEQ = 6
G4 = 24                      # 4*OUT gates

_cached = {}


def build_program(debug_taps=False):
    nc = bacc.Bacc("TRN2", target_bir_lowering=False, debug=False,
                   enable_asserts=False, num_devices=NCORES)

    # ---- kernel I/O ----
    enc_d = nc.dram_tensor("enc", [NS, KDIM], F32, kind="ExternalInput")
    wta_d = nc.dram_tensor("wta", [19, NS], BF16, kind="ExternalInput")
    ait_d = nc.dram_tensor("ait", [19, 6], BF16, kind="ExternalInput")
    w2t_d = nc.dram_tensor("w2t", [128, NT * G4], BF16, kind="ExternalInput")
    gob_d = nc.dram_tensor("gob", [6, G4], F32, kind="ExternalInput")
    whht_d = nc.dram_tensor("whht", [6, G4], BF16, kind="ExternalInput")
    h0t_d = nc.dram_tensor("h0t", [6, 1], BF16, kind="ExternalInput")
    c0_d = nc.dram_tensor("c0", [1, 6], F32, kind="ExternalInput")
    w11t_d = nc.dram_tensor("w11t", [6, HIDDEN], BF16, kind="ExternalInput")
    b11r_d = nc.dram_tensor("b11r", [128, 384], F32, kind="ExternalInput")
    w12t_d = nc.dram_tensor("w12t", [128, 384], BF16, kind="ExternalInput")
    b12r_d = nc.dram_tensor("b12r", [6, 6], F32, kind="ExternalInput")
    iden_d = nc.dram_tensor("iden", [128, 128], F32, kind="ExternalInput")

    logp_d = nc.dram_tensor("logp", [6, 6], F32, kind="ExternalOutput")
    hout_d = nc.dram_tensor("hout", [1, 6], F32, kind="ExternalOutput")
    cout_d = nc.dram_tensor("cout", [1, 6], F32, kind="ExternalOutput")
    warm_d = nc.dram_tensor("warm", [1, 2], F32, kind="ExternalOutput")
    if debug_taps:
        det_d = nc.dram_tensor("d_et", [128, HC * 6], F32, kind="ExternalOutput")
        dat_d = nc.dram_tensor("d_at", [128, NT * 6], F32, kind="ExternalOutput")
        dred_d = nc.dram_tensor("d_red", [25, 6], F32, kind="ExternalOutput")
        dgt_d = nc.dram_tensor("d_gt", [G4, 6], F32, kind="ExternalOutput")
        dhs_d = nc.dram_tensor("d_hs", [6, SEQ], F32, kind="ExternalOutput")
        dy1_d = nc.dram_tensor("d_y1", [128, 384], F32, kind="ExternalOutput")
        dpay_d = nc.dram_tensor("d_pay", [25, 6], F32, kind="ExternalOutput")

    with tile.TileContext(nc) as tc:
        with (
            tc.tile_pool(name="consts", bufs=1) as cp,
            tc.tile_pool(name="encp", bufs=3) as encp,
            tc.tile_pool(name="work", bufs=2) as wp,
            tc.tile_pool(name="ps1", bufs=1, space=bass.MemorySpace.PSUM) as ps1,
            tc.tile_pool(name="dram", bufs=1, space="DRAM") as dp,
        ):
            # ---- load constants ----
            wta = cp.tile([19, NS], BF16)
            nc.sync.dma_start(wta[:], wta_d[:])
            ait = cp.tile([19, 6], BF16)
            nc.sync.dma_start(ait[:], ait_d[:])
            w2t = cp.tile([128, NT * G4], BF16)
            nc.sync.dma_start(w2t[:], w2t_d[:])
            gob = cp.tile([6, G4], F32)
            nc.sync.dma_start(gob[:], gob_d[:])
            whht = cp.tile([6, G4], BF16)
            nc.sync.dma_start(whht[:], whht_d[:])
            h0t = cp.tile([6, 1], BF16)
            nc.sync.dma_start(h0t[:], h0t_d[:])
            c0 = cp.tile([1, 6], F32)
            nc.sync.dma_start(c0[:], c0_d[:])
            w11t = cp.tile([6, HIDDEN], BF16)
            nc.sync.dma_start(w11t[:], w11t_d[:])
            b11r = cp.tile([128, 384], F32)
            nc.sync.dma_start(b11r[:], b11r_d[:])
            w12t = cp.tile([128, 384], BF16)
            nc.sync.dma_start(w12t[:], w12t_d[:])
            b12r = cp.tile([6, 6], F32)
            nc.sync.dma_start(b12r[:], b12r_d[:])
            iden = cp.tile([128, 128], F32)
            nc.sync.dma_start(iden[:], iden_d[:])
            ones = cp.tile([128, 1], BF16)
            nc.vector.memset(ones[:], 1.0)

            # warm up the collectives firmware so the real AllReduce
            # does not pay the cold ncfw trigger latency
            warm_sb = cp.tile([1, 2], F32)
            nc.vector.memset(warm_sb[:], 0.0)
            war_in = dp.tile([1, 2], F32)
            war_out = dp.tile([1, 2], F32)
            # scalar-queue DMAs: keep the sync HWDGE ring free for the enc
            # stream (the warm_d DMA blocks its queue until the collective
            # completes)
            nc.scalar.dma_start(war_in[:], warm_sb[:])
            nc.gpsimd.collective_compute(
                "AllReduce", mybir.AluOpType.add,
                replica_groups=[list(range(NCORES))],
                ins=[war_in.opt()], outs=[war_out.opt()],
            )
            nc.scalar.dma_start(warm_d[:], war_out[:])

            # ---- attention logits (transposed layout) + exp ----
            # lt[:, 6m:6m+6] = W_attn_shard[128m:128m+128, :] @ attn_in.T + b  (bias folded via ones row)
            lt = ps1.tile([128, HC * 6], F32, tag="pa")
            for m in range(HC):
                nc.tensor.matmul(lt[:, 6 * m:6 * m + 6],
                                 wta[:, 128 * m:128 * (m + 1)], ait[:],
                                 start=True, stop=True)
            et = cp.tile([128, HC * 6], BF16)   # exp(logits), chunked transposed
            nc.scalar.activation(et[:], lt[:], mybir.ActivationFunctionType.Exp)

            # ---- big matmul: appliedT[n, r] = sum_h enc[h, n] * et[h, r] ----
            at_ps = ps1.tile([128, NT * 6], F32, tag="pa")
            s_ps = ps1.tile([1, 6], F32, tag="sp")       # partial softmax denominator
            for m in range(HC):
                encf = encp.tile([128, KDIM], F32, tag="encf")
                nc.sync.dma_start(encf[:], enc_d[128 * m:128 * (m + 1), :])
                enct = encp.tile([128, KDIM], BF16, tag="enct")
                nc.vector.tensor_copy(enct[:], encf[:])
                etm = et[:, 6 * m:6 * m + 6]
                for j in range(NT):
                    w = 128 if j < NT - 1 else KDIM - 128 * (NT - 1)
                    # start=True only on the very first matmul into this bank:
                    # PE's has_written clear is bank-wide, so later groups must
                    # rely on "overwrite where bit is clear" for their first
                    # write and accumulate afterwards.
                    nc.tensor.matmul(at_ps[0:w, 6 * j:6 * j + 6],
                                     enct[:, 128 * j:128 * j + w], etm,
                                     start=(m == 0 and j == 0),
                                     stop=(m == HC - 1 and j == NT - 1))
                nc.tensor.matmul(s_ps[:], ones[:], etm,
                                 start=(m == 0), stop=(m == HC - 1))

            at_sb = cp.tile([128, NT * 6], BF16)
            nc.vector.tensor_copy(at_sb[:], at_ps[:])

            # ---- partial gate pre-activations: gT[j, r] = sum_n appliedT[n,r] W2T[n,j] ----
            gt_ps = ps1.tile([G4, 6], F32, tag="gt")
            for j in range(NT):
                w = 128 if j < NT - 1 else KDIM - 128 * (NT - 1)
                nc.tensor.matmul(gt_ps[:], w2t[0:w, G4 * j:G4 * (j + 1)],
                                 at_sb[0:w, 6 * j:6 * j + 6],
                                 start=(j == 0), stop=(j == NT - 1))

            g24 = cp.tile([G4, 6], F32)
            nc.vector.tensor_copy(g24[:], gt_ps[:])
            s_sb = cp.tile([1, 6], F32)
            nc.vector.tensor_copy(s_sb[:], s_ps[:])

            # ---- AllReduce of the (25, 6) partial ----
            ar_in = dp.tile([25, 6], F32)
            ar_out = dp.tile([25, 6], F32)
            nc.sync.dma_start(ar_in[0:G4, :], g24[:])
            nc.sync.dma_start(ar_in[G4:G4 + 1, :], s_sb[:])
            nc.gpsimd.collective_compute(
                "AllReduce", mybir.AluOpType.add,
                replica_groups=[list(range(NCORES))],
                ins=[ar_in.opt()], outs=[ar_out.opt()],
            )
            red = cp.tile([25, 6], F32)
            nc.sync.dma_start(red[:], ar_out[:])

            # ---- finish gates: (6,24) row layout ----
            redT_ps = ps1.tile([6, 25], F32, tag="sp")
            nc.tensor.transpose(redT_ps[:], red[:], iden[0:25, 0:25])
            redT = cp.tile([6, 25], F32)
            nc.vector.tensor_copy(redT[:], redT_ps[:])
            inv_s = cp.tile([6, 1], F32)
            nc.vector.reciprocal(inv_s[:], redT[:, G4:G4 + 1])
            grow = cp.tile([6, G4], F32)
            # grow = redT[:, :24] * inv_s + gob
            nc.vector.scalar_tensor_tensor(grow[:], redT[:, 0:G4], inv_s[:],
                                           gob[:], mybir.AluOpType.mult,
                                           mybir.AluOpType.add)
            gT_ps = ps1.tile([G4, 6], F32, tag="gt")
            nc.tensor.transpose(gT_ps[:], grow[:], iden[0:6, 0:6])
            gT = cp.tile([G4, 6], F32)
            nc.vector.tensor_copy(gT[:], gT_ps[:])

            # ---- LSTM (6 steps, tiny) ----
            hsT = cp.tile([6, SEQ], BF16)
            h_prev_t = h0t
            c_prev = c0
            h_new = None
            c_new = None
            for t in range(SEQ):
                wh_ps = ps1.tile([G4, 1], F32, tag="wh")
                nc.tensor.matmul(wh_ps[:], whht[:], h_prev_t[:],
                                 start=True, stop=True)
                gcol = wp.tile([G4, 1], F32)
                nc.vector.tensor_add(gcol[:], gT[:, t:t + 1], wh_ps[:])
                gr_ps = ps1.tile([1, G4], F32, tag="gr")
                nc.tensor.transpose(gr_ps[:], gcol[:], iden[0:G4, 0:G4])
                sig = wp.tile([1, G4], F32)
                nc.scalar.activation(sig[:], gr_ps[:],
                                     mybir.ActivationFunctionType.Sigmoid)
                tng = wp.tile([1, 6], F32)
                nc.scalar.activation(tng[:], gr_ps[:, 12:18],
                                     mybir.ActivationFunctionType.Tanh)
                m1 = wp.tile([1, 6], F32)
                nc.vector.tensor_mul(m1[:], sig[:, 6:12], c_prev[:])
                m2 = wp.tile([1, 6], F32)
                nc.vector.tensor_mul(m2[:], sig[:, 0:6], tng[:])
                c_new = wp.tile([1, 6], F32)
                nc.vector.tensor_add(c_new[:], m1[:], m2[:])
                tc_ = wp.tile([1, 6], F32)
                nc.scalar.activation(tc_[:], c_new[:],
                                     mybir.ActivationFunctionType.Tanh)
                h_new = wp.tile([1, 6], F32)
                nc.vector.tensor_mul(h_new[:], sig[:, 18:24], tc_[:])
                ht_ps = ps1.tile([6, 1], F32, tag="ht")
                nc.tensor.transpose(ht_ps[:], h_new[:], iden[0:1, 0:1])
                nc.vector.tensor_copy(hsT[:, t:t + 1], ht_ps[:])
                h_prev_t = hsT[:, t:t + 1]
                c_prev = c_new

            nc.sync.dma_start(hout_d[:], h_new[:])
            nc.sync.dma_start(cout_d[:], c_new[:])

            # ---- output head ----
            y1_ps = ps1.tile([128, 384], F32, tag="pa")
            for cch in range(64):
                nc.tensor.matmul(y1_ps[:, 6 * cch:6 * cch + 6],
                                 w11t[:, 128 * cch:128 * (cch + 1)], hsT[:],
                                 start=True, stop=True)
            y1b = cp.tile([128, 384], F32)
            nc.vector.tensor_add(y1b[:], y1_ps[:], b11r[:])
            y1t = cp.tile([128, 384], BF16)
            nc.vector.tensor_relu(y1t[:], y1b[:])

            y2_ps = ps1.tile([6, 6], F32, tag="sp")
            for cch in range(64):
                nc.tensor.matmul(y2_ps[:], y1t[:, 6 * cch:6 * cch + 6],
                                 w12t[:, 6 * cch:6 * cch + 6],
                                 start=(cch == 0), stop=(cch == 63))
            y2b = cp.tile([6, 6], F32)
            nc.vector.tensor_add(y2b[:], y2_ps[:], b12r[:])
            y2r = cp.tile([6, 6], F32)
            nc.vector.tensor_relu(y2r[:], y2b[:])

            # ---- log_softmax over free axis ----
            mx = cp.tile([6, 1], F32)
            nc.vector.tensor_reduce(mx[:], y2r[:], mybir.AxisListType.X,
                                    mybir.AluOpType.max)
            nmx = cp.tile([6, 1], F32)
            nc.vector.tensor_scalar_mul(nmx[:], mx[:], -1.0)
            e = cp.tile([6, 6], F32)
            se = cp.tile([6, 1], F32)
            nc.scalar.activation(e[:], y2r[:], mybir.ActivationFunctionType.Exp,
                                 bias=nmx[:], accum_out=se[:])
            lse = cp.tile([6, 1], F32)
            nc.scalar.activation(lse[:], se[:], mybir.ActivationFunctionType.Ln)
            shift = cp.tile([6, 1], F32)
            nc.vector.tensor_sub(shift[:], nmx[:], lse[:])
            logp_sb = cp.tile([6, 6], F32)
            nc.vector.tensor_scalar_add(logp_sb[:], y2r[:], shift[:])
            nc.sync.dma_start(logp_d[:], logp_sb[:])

            if debug_taps:
                nc.sync.dma_start(det_d[:], et[:])
                nc.sync.dma_start(dat_d[:], at_sb[:])
                nc.sync.dma_start(dred_d[:], red[:])
                nc.sync.dma_start(dgt_d[:], gT[:])
                nc.sync.dma_start(dhs_d[:], hsT[:])
                nc.sync.dma_start(dy1_d[:], y1t[:])
                nc.sync.dma_start(dpay_d[0:G4, :], g24[:])
                nc.sync.dma_start(dpay_d[G4:G4 + 1, :], s_sb[:])

    nc.compile()
    return nc


def prep_inputs(inp, hn, cn, encoder_outputs, W_attn, b_attn, W_ih, W_hh,
                b_ih, b_hh, W_out11, b_out11, W_out12, b_out12):
    f32 = np.float32
    inp = np.asarray(inp).astype(np.int64)
    hn = np.asarray(hn, f32).reshape(6)
    cn = np.asarray(cn, f32).reshape(6)
    enc2d = np.asarray(encoder_outputs, f32).reshape(HIDDEN, KDIM)
    W_attn = np.asarray(W_attn, f32)
    b_attn = np.asarray(b_attn, f32)
    W_ih = np.asarray(W_ih, f32)
    W_hh = np.asarray(W_hh, f32)
    b_ih = np.asarray(b_ih, f32)
    b_hh = np.asarray(b_hh, f32)
    W_out11 = np.asarray(W_out11, f32)
    b_out11 = np.asarray(b_out11, f32)
    W_out12 = np.asarray(W_out12, f32)
    b_out12 = np.asarray(b_out12, f32)

    oh = np.eye(6, dtype=f32)[inp]                                   # (6,6)
    attn_in = np.concatenate(
        [oh, np.broadcast_to(hn, (6, 6)), np.broadcast_to(cn, (6, 6))], axis=1)
    ait = np.ascontiguousarray(
        np.concatenate([attn_in.T, np.ones((1, 6), f32)], axis=0)).astype(BF)

    W2T = W_ih[:, 6:].T                                              # (4114,24)
    W2T_pad = np.zeros((NT * 128, G4), f32)
    W2T_pad[:KDIM] = W2T
    w2t = np.ascontiguousarray(
        W2T_pad.reshape(NT, 128, G4).transpose(1, 0, 2)
        .reshape(128, NT * G4)).astype(BF)

    gob = np.ascontiguousarray(oh @ W_ih[:, :6].T + b_ih + b_hh)     # (6,24)
    whht = np.ascontiguousarray(W_hh.T).astype(BF)                   # (6,24)
    h0t = np.ascontiguousarray(hn.reshape(6, 1)).astype(BF)
    c0 = np.ascontiguousarray(cn.reshape(1, 6))
    w11t = np.ascontiguousarray(W_out11.T).astype(BF)                # (6,8192)
    b11r = np.ascontiguousarray(
        np.repeat(b_out11.reshape(64, 128).T.reshape(128, 64, 1), 6,
                  axis=2).reshape(128, 384))
    w12t = np.ascontiguousarray(
        W_out12.T.reshape(64, 128, 6).transpose(1, 0, 2)
        .reshape(128, 384)).astype(BF)
    b12r = np.ascontiguousarray(np.broadcast_to(b_out12, (6, 6)))
    iden = np.eye(128, dtype=f32)

    shared = dict(ait=ait, w2t=w2t, gob=gob, whht=whht, h0t=h0t, c0=c0,
                  w11t=w11t, b11r=b11r, w12t=w12t, b12r=b12r, iden=iden)
    in_maps = []
    for c in range(NCORES):
        lo = c * NS
        wta = np.ascontiguousarray(np.concatenate(
            [W_attn[lo:lo + NS].T, b_attn[None, lo:lo + NS]],
            axis=0)).astype(BF)
        m = dict(shared)
        m["enc"] = enc2d[lo:lo + NS]
        m["wta"] = wta
        in_maps.append(m)
    return in_maps


def kernel(**inputs):
    if "nc" not in _cached:
        _cached["nc"] = build_program()
    nc = _cached["nc"]
    in_maps = prep_inputs(**inputs)
    res = run_bass_kernel_spmd(nc, in_maps, core_ids=list(range(NCORES)))
    out = res.results[0]
    logp = np.asarray(out["logp"], np.float32)
    h = np.asarray(out["hout"], np.float32).reshape(1, 1, 6)
    c = np.asarray(out["cout"], np.float32).reshape(1, 1, 6)
    return (logp, h, c)


# revision 13
# speedup vs baseline: 1.2617x; 1.1430x over previous
"""Trainium2 Bass kernel for nn_AttentionRevisedDecoderRNN.

Computation (batch=1, seq=6):
  attn_w  = softmax(attn_in @ W_attn.T + b_attn, axis=hidden)      # (6, 8192)
  applied = attn_w @ enc                                           # (6, 4114)  <- memory bound
  x       = [one_hot, applied]; LSTM(hidden=6) over 6 steps
  y       = relu(relu(hs @ W_out11.T + b11) @ W_out12.T + b12); log_softmax

Strategy: shard enc + W_attn along hidden (8192 -> 1024/core) over 8 cores.
Each core computes exp(logits) for its hidden shard (softmax numerators),
contracts them against its enc shard into a partial appliedT, then folds the
LSTM input projection (applied @ W_ih[:,6:].T) and the softmax denominator s
into a tiny (25,6) partial that is AllReduced.  After the AllReduce every
core runs the (tiny) LSTM + output head redundantly; core 0's output is used.
"""

import sys
import numpy as np
import ml_dtypes

BF = ml_dtypes.bfloat16

for _p in ("/opt/trn_rl_repo",):
    if _p not in sys.path:
        sys.path.insert(0, _p)

from concourse import bass, bacc, tile, mybir
from concourse.bass_utils import run_bass_kernel_spmd

F32 = mybir.dt.float32
BF16 = mybir.dt.bfloat16
NCORES = 8
HIDDEN = 8192
NS = HIDDEN // NCORES        # hidden shard per core = 1024
HC = NS // 128               # h-chunks per core = 8
KDIM = 4114                  # enc free width
NT = (KDIM + 127) // 128     # 33 n-tiles (last = 18 rows)
SEQ = 6
G4 = 24                      # 4*OUT gates

_cached = {}


def build_program(debug_taps=False):
    nc = bacc.Bacc("TRN2", target_bir_lowering=False, debug=False,
                   enable_asserts=False, num_devices=NCORES)

    # ---- kernel I/O ----
    enc_d = nc.dram_tensor("enc", [NS, KDIM], F32, kind="ExternalInput")
    wta_d = nc.dram_tensor("wta", [19, NS], BF16, kind="ExternalInput")
    ait_d = nc.dram_tensor("ait", [19, 6], BF16, kind="ExternalInput")
    w2t_d = nc.dram_tensor("w2t", [128, NT * G4], BF16, kind="ExternalInput")
    gob_d = nc.dram_tensor("gob", [6, G4], F32, kind="ExternalInput")
    whht_d = nc.dram_tensor("whht", [6, G4], BF16, kind="ExternalInput")
    h0t_d = nc.dram_tensor("h0t", [6, 1], BF16, kind="ExternalInput")
    c0_d = nc.dram_tensor("c0", [1, 6], F32, kind="ExternalInput")
    w11t_d = nc.dram_tensor("w11t", [6, HIDDEN], BF16, kind="ExternalInput")
    b11r_d = nc.dram_tensor("b11r", [128, 384], F32, kind="ExternalInput")
    w12t_d = nc.dram_tensor("w12t", [128, 384], BF16, kind="ExternalInput")
    b12r_d = nc.dram_tensor("b12r", [6, 6], F32, kind="ExternalInput")
    iden_d = nc.dram_tensor("iden", [128, 128], F32, kind="ExternalInput")

    logp_d = nc.dram_tensor("logp", [6, 6], F32, kind="ExternalOutput")
    hout_d = nc.dram_tensor("hout", [1, 6], F32, kind="ExternalOutput")
    cout_d = nc.dram_tensor("cout", [1, 6], F32, kind="ExternalOutput")
    warm_d = nc.dram_tensor("warm", [1, 2], F32, kind="ExternalOutput")
    if debug_taps:
        det_d = nc.dram_tensor("d_et", [128, HC * 6], F32, kind="ExternalOutput")
        dat_d = nc.dram_tensor("d_at", [128, NT * 6], F32, kind="ExternalOutput")
        dred_d = nc.dram_tensor("d_red", [25, 6], F32, kind="ExternalOutput")
        dgt_d = nc.dram_tensor("d_gt", [G4, 6], F32, kind="ExternalOutput")
        dhs_d = nc.dram_tensor("d_hs", [6, SEQ], F32, kind="ExternalOutput")
        dy1_d = nc.dram_tensor("d_y1", [128, 384], F32, kind="ExternalOutput")
        dpay_d = nc.dram_tensor("d_pay", [25, 6], F32, kind="ExternalOutput")

    with tile.TileContext(nc) as tc:
        with (
            tc.tile_pool(name="consts", bufs=1) as cp,
            tc.tile_pool(name="encp", bufs=3) as encp,
            tc.tile_pool(name="work", bufs=2) as wp,
            tc.tile_pool(name="ps1", bufs=1, space=bass.MemorySpace.PSUM) as ps1,
            tc.tile_pool(name="dram", bufs=1, space="DRAM") as dp,
        ):
            # ---- load constants ----
            wta = cp.tile([19, NS], BF16)
            nc.scalar.dma_start(wta[:], wta_d[:])
            ait = cp.tile([19, 6], BF16)
            nc.scalar.dma_start(ait[:], ait_d[:])
            w2t = cp.tile([128, NT * G4], BF16)
            nc.scalar.dma_start(w2t[:], w2t_d[:])
            gob = cp.tile([6, G4], F32)
            nc.scalar.dma_start(gob[:], gob_d[:])
            whht = cp.tile([6, G4], BF16)
            nc.scalar.dma_start(whht[:], whht_d[:])
            h0t = cp.tile([6, 1], BF16)
            nc.scalar.dma_start(h0t[:], h0t_d[:])
            c0 = cp.tile([1, 6], F32)
            nc.scalar.dma_start(c0[:], c0_d[:])
            w11t = cp.tile([6, HIDDEN], BF16)
            nc.scalar.dma_start(w11t[:], w11t_d[:])
            b11r = cp.tile([128, 384], F32)
            nc.scalar.dma_start(b11r[:], b11r_d[:])
            w12t = cp.tile([128, 384], BF16)
            nc.scalar.dma_start(w12t[:], w12t_d[:])
            b12r = cp.tile([6, 6], F32)
            nc.scalar.dma_start(b12r[:], b12r_d[:])
            iden = cp.tile([128, 128], F32)
            nc.scalar.dma_start(iden[:], iden_d[:])
            ones = cp.tile([128, 1], BF16)
            nc.vector.memset(ones[:], 1.0)

            # warm up the collectives firmware so the real AllReduce
            # does not pay the cold ncfw trigger latency
            warm_sb = cp.tile([1, 2], F32)
            nc.vector.memset(warm_sb[:], 0.0)
            war_in = dp.tile([1, 2], F32)
            war_out = dp.tile([1, 2], F32)
            # scalar-queue DMAs: keep the sync HWDGE ring free for the enc
            # stream (the warm_d DMA blocks its queue until the collective
            # completes)
            nc.scalar.dma_start(war_in[:], warm_sb[:])
            nc.gpsimd.collective_compute(
                "AllReduce", mybir.AluOpType.add,
                replica_groups=[list(range(NCORES))],
                ins=[war_in.opt()], outs=[war_out.opt()],
            )
            nc.scalar.dma_start(warm_d[:], war_out[:])

            # ---- attention logits (transposed layout) + exp ----
            # lt[:, 6m:6m+6] = W_attn_shard[128m:128m+128, :] @ attn_in.T + b  (bias folded via ones row)
            lt = ps1.tile([128, HC * 6], F32, tag="pa")
            for m in range(HC):
                nc.tensor.matmul(lt[:, 6 * m:6 * m + 6],
                                 wta[:, 128 * m:128 * (m + 1)], ait[:],
                                 start=True, stop=True)
            et = cp.tile([128, HC * 6], BF16)   # exp(logits), chunked transposed
            nc.scalar.activation(et[:], lt[:], mybir.ActivationFunctionType.Exp)

            # ---- big matmul: appliedT[n, r] = sum_h enc[h, n] * et[h, r] ----
            at_ps = ps1.tile([128, NT * 6], F32, tag="pa")
            s_ps = ps1.tile([1, 6], F32, tag="sp")       # partial softmax denominator
            for m in range(HC):
                encf = encp.tile([128, KDIM], F32, tag="encf")
                nc.sync.dma_start(encf[:], enc_d[128 * m:128 * (m + 1), :])
                enct = encp.tile([128, KDIM], BF16, tag="enct")
                nc.vector.tensor_copy(enct[:], encf[:])
                etm = et[:, 6 * m:6 * m + 6]
                for j in range(NT):
                    w = 128 if j < NT - 1 else KDIM - 128 * (NT - 1)
                    # start=True only on the very first matmul into this bank:
                    # PE's has_written clear is bank-wide, so later groups must
                    # rely on "overwrite where bit is clear" for their first
                    # write and accumulate afterwards.
                    nc.tensor.matmul(at_ps[0:w, 6 * j:6 * j + 6],
                                     enct[:, 128 * j:128 * j + w], etm,
                                     start=(m == 0 and j == 0),
                                     stop=(m == HC - 1 and j == NT - 1))
                nc.tensor.matmul(s_ps[:], ones[:], etm,
                                 start=(m == 0), stop=(m == HC - 1))

            at_sb = cp.tile([128, NT * 6], BF16)
            nc.vector.tensor_copy(at_sb[:], at_ps[:])

            # ---- partial gate pre-activations: gT[j, r] = sum_n appliedT[n,r] W2T[n,j] ----
            gt_ps = ps1.tile([G4, 6], F32, tag="gt")
            for j in range(NT):
                w = 128 if j < NT - 1 else KDIM - 128 * (NT - 1)
                nc.tensor.matmul(gt_ps[:], w2t[0:w, G4 * j:G4 * (j + 1)],
                                 at_sb[0:w, 6 * j:6 * j + 6],
                                 start=(j == 0), stop=(j == NT - 1))

            g24 = cp.tile([G4, 6], F32)
            nc.vector.tensor_copy(g24[:], gt_ps[:])
            s_sb = cp.tile([1, 6], F32)
            nc.vector.tensor_copy(s_sb[:], s_ps[:])

            # ---- AllReduce of the (25, 6) partial ----
            ar_in = dp.tile([25, 6], F32)
            ar_out = dp.tile([25, 6], F32)
            nc.sync.dma_start(ar_in[0:G4, :], g24[:])
            nc.sync.dma_start(ar_in[G4:G4 + 1, :], s_sb[:])
            nc.gpsimd.collective_compute(
                "AllReduce", mybir.AluOpType.add,
                replica_groups=[list(range(NCORES))],
                ins=[ar_in.opt()], outs=[ar_out.opt()],
            )
            red = cp.tile([25, 6], F32)
            nc.sync.dma_start(red[:], ar_out[:])

            # ---- finish gates: (6,24) row layout ----
            redT_ps = ps1.tile([6, 25], F32, tag="sp")
            nc.tensor.transpose(redT_ps[:], red[:], iden[0:25, 0:25])
            redT = cp.tile([6, 25], F32)
            nc.vector.tensor_copy(redT[:], redT_ps[:])
            inv_s = cp.tile([6, 1], F32)
            nc.vector.reciprocal(inv_s[:], redT[:, G4:G4 + 1])
            grow = cp.tile([6, G4], F32)
            # grow = redT[:, :24] * inv_s + gob
            nc.vector.scalar_tensor_tensor(grow[:], redT[:, 0:G4], inv_s[:],
                                           gob[:], mybir.AluOpType.mult,
                                           mybir.AluOpType.add)
            gT_ps = ps1.tile([G4, 6], F32, tag="gt")
            nc.tensor.transpose(gT_ps[:], grow[:], iden[0:6, 0:6])
            gT = cp.tile([G4, 6], F32)
            nc.vector.tensor_copy(gT[:], gT_ps[:])

            # ---- LSTM (6 steps, tiny) ----
            hsT = cp.tile([6, SEQ], BF16)
            h_prev_t = h0t
            c_prev = c0
            h_new = None
            c_new = None
            for t in range(SEQ):
                wh_ps = ps1.tile([G4, 1], F32, tag="wh")
                nc.tensor.matmul(wh_ps[:], whht[:], h_prev_t[:],
                                 start=True, stop=True)
                gcol = wp.tile([G4, 1], F32)
                nc.vector.tensor_add(gcol[:], gT[:, t:t + 1], wh_ps[:])
                gr_ps = ps1.tile([1, G4], F32, tag="gr")
                nc.tensor.transpose(gr_ps[:], gcol[:], iden[0:G4, 0:G4])
                sig = wp.tile([1, G4], F32)
                nc.scalar.activation(sig[:], gr_ps[:],
                                     mybir.ActivationFunctionType.Sigmoid)
                tng = wp.tile([1, 6], F32)
                nc.scalar.activation(tng[:], gr_ps[:, 12:18],
                                     mybir.ActivationFunctionType.Tanh)
                m1 = wp.tile([1, 6], F32)
                nc.vector.tensor_mul(m1[:], sig[:, 6:12], c_prev[:])
                m2 = wp.tile([1, 6], F32)
                nc.vector.tensor_mul(m2[:], sig[:, 0:6], tng[:])
                c_new = wp.tile([1, 6], F32)
                nc.vector.tensor_add(c_new[:], m1[:], m2[:])
                tc_ = wp.tile([1, 6], F32)
                nc.scalar.activation(tc_[:], c_new[:],
                                     mybir.ActivationFunctionType.Tanh)
                h_new = wp.tile([1, 6], F32)
                nc.vector.tensor_mul(h_new[:], sig[:, 18:24], tc_[:])
                ht_ps = ps1.tile([6, 1], F32, tag="ht")
                nc.tensor.transpose(ht_ps[:], h_new[:], iden[0:1, 0:1])
                nc.vector.tensor_copy(hsT[:, t:t + 1], ht_ps[:])
                h_prev_t = hsT[:, t:t + 1]
                c_prev = c_new

            nc.sync.dma_start(hout_d[:], h_new[:])
            nc.sync.dma_start(cout_d[:], c_new[:])

            # ---- output head ----
            y1_ps = ps1.tile([128, 384], F32, tag="pa")
            for cch in range(64):
                nc.tensor.matmul(y1_ps[:, 6 * cch:6 * cch + 6],
                                 w11t[:, 128 * cch:128 * (cch + 1)], hsT[:],
                                 start=True, stop=True)
            y1b = cp.tile([128, 384], F32)
            nc.vector.tensor_add(y1b[:], y1_ps[:], b11r[:])
            y1t = cp.tile([128, 384], BF16)
            nc.vector.tensor_relu(y1t[:], y1b[:])

            y2_ps = ps1.tile([6, 6], F32, tag="sp")
            for cch in range(64):
                nc.tensor.matmul(y2_ps[:], y1t[:, 6 * cch:6 * cch + 6],
                                 w12t[:, 6 * cch:6 * cch + 6],
                                 start=(cch == 0), stop=(cch == 63))
            y2b = cp.tile([6, 6], F32)
            nc.vector.tensor_add(y2b[:], y2_ps[:], b12r[:])
            y2r = cp.tile([6, 6], F32)
            nc.vector.tensor_relu(y2r[:], y2b[:])

            # ---- log_softmax over free axis ----
            mx = cp.tile([6, 1], F32)
            nc.vector.tensor_reduce(mx[:], y2r[:], mybir.AxisListType.X,
                                    mybir.AluOpType.max)
            nmx = cp.tile([6, 1], F32)
            nc.vector.tensor_scalar_mul(nmx[:], mx[:], -1.0)
            e = cp.tile([6, 6], F32)
            se = cp.tile([6, 1], F32)
            nc.scalar.activation(e[:], y2r[:], mybir.ActivationFunctionType.Exp,
                                 bias=nmx[:], accum_out=se[:])
            lse = cp.tile([6, 1], F32)
            nc.scalar.activation(lse[:], se[:], mybir.ActivationFunctionType.Ln)
            shift = cp.tile([6, 1], F32)
            nc.vector.tensor_sub(shift[:], nmx[:], lse[:])
            logp_sb = cp.tile([6, 6], F32)
            nc.vector.tensor_scalar_add(logp_sb[:], y2r[:], shift[:])
            nc.sync.dma_start(logp_d[:], logp_sb[:])

            if debug_taps:
                nc.sync.dma_start(det_d[:], et[:])
                nc.sync.dma_start(dat_d[:], at_sb[:])
                nc.sync.dma_start(dred_d[:], red[:])
                nc.sync.dma_start(dgt_d[:], gT[:])
                nc.sync.dma_start(dhs_d[:], hsT[:])
                nc.sync.dma_start(dy1_d[:], y1t[:])
                nc.sync.dma_start(dpay_d[0:G4, :], g24[:])
                nc.sync.dma_start(dpay_d[G4:G4 + 1, :], s_sb[:])

    nc.compile()
    return nc


def prep_inputs(inp, hn, cn, encoder_outputs, W_attn, b_attn, W_ih, W_hh,
                b_ih, b_hh, W_out11, b_out11, W_out12, b_out12):
    f32 = np.float32
    inp = np.asarray(inp).astype(np.int64)
    hn = np.asarray(hn, f32).reshape(6)
    cn = np.asarray(cn, f32).reshape(6)
    enc2d = np.asarray(encoder_outputs, f32).reshape(HIDDEN, KDIM)
    W_attn = np.asarray(W_attn, f32)
    b_attn = np.asarray(b_attn, f32)
    W_ih = np.asarray(W_ih, f32)
    W_hh = np.asarray(W_hh, f32)
    b_ih = np.asarray(b_ih, f32)
    b_hh = np.asarray(b_hh, f32)
    W_out11 = np.asarray(W_out11, f32)
    b_out11 = np.asarray(b_out11, f32)
    W_out12 = np.asarray(W_out12, f32)
    b_out12 = np.asarray(b_out12, f32)

    oh = np.eye(6, dtype=f32)[inp]                                   # (6,6)
    attn_in = np.concatenate(
        [oh, np.broadcast_to(hn, (6, 6)), np.broadcast_to(cn, (6, 6))], axis=1)
    ait = np.ascontiguousarray(
        np.concatenate([attn_in.T, np.ones((1, 6), f32)], axis=0)).astype(BF)

    W2T = W_ih[:, 6:].T                                              # (4114,24)
    W2T_pad = np.zeros((NT * 128, G4), f32)
    W2T_pad[:KDIM] = W2T
    w2t = np.ascontiguousarray(
        W2T_pad.reshape(NT, 128, G4).transpose(1, 0, 2)
        .reshape(128, NT * G4)).astype(BF)

    gob = np.ascontiguousarray(oh @ W_ih[:, :6].T + b_ih + b_hh)     # (6,24)
    whht = np.ascontiguousarray(W_hh.T).astype(BF)                   # (6,24)
    h0t = np.ascontiguousarray(hn.reshape(6, 1)).astype(BF)
    c0 = np.ascontiguousarray(cn.reshape(1, 6))
    w11t = np.ascontiguousarray(W_out11.T).astype(BF)                # (6,8192)
    b11r = np.ascontiguousarray(
        np.repeat(b_out11.reshape(64, 128).T.reshape(128, 64, 1), 6,
                  axis=2).reshape(128, 384))
    w12t = np.ascontiguousarray(
        W_out12.T.reshape(64, 128, 6).transpose(1, 0, 2)
        .reshape(128, 384)).astype(BF)
    b12r = np.ascontiguousarray(np.broadcast_to(b_out12, (6, 6)))
    iden = np.eye(128, dtype=f32)

    shared = dict(ait=ait, w2t=w2t, gob=gob, whht=whht, h0t=h0t, c0=c0,
                  w11t=w11t, b11r=b11r, w12t=w12t, b12r=b12r, iden=iden)
    in_maps = []
    for c in range(NCORES):
        lo = c * NS
        wta = np.ascontiguousarray(np.concatenate(
            [W_attn[lo:lo + NS].T, b_attn[None, lo:lo + NS]],
            axis=0)).astype(BF)
        m = dict(shared)
        m["enc"] = enc2d[lo:lo + NS]
        m["wta"] = wta
        in_maps.append(m)
    return in_maps


def kernel(**inputs):
    if "nc" not in _cached:
        _cached["nc"] = build_program()
    nc = _cached["nc"]
    in_maps = prep_inputs(**inputs)
    res = run_bass_kernel_spmd(nc, in_maps, core_ids=list(range(NCORES)))
    out = res.results[0]
    logp = np.asarray(out["logp"], np.float32)
    h = np.asarray(out["hout"], np.float32).reshape(1, 1, 6)
    c = np.asarray(out["cout"], np.float32).reshape(1, 1, 6)
    return (logp, h, c)
